# revision 34
# baseline (speedup 1.0000x reference)
"""BitLinear (BitNet 1.58-bit ternary) distributed Trainium2 kernel.

Reference semantics:
    scale = max(mean(|w|), 1e-5)
    w_q   = sign(w) * (|w| > scale/3)          # ternary {-1, 0, 1}
    out   = (x @ w_q.T) * scale                # x: [4, 2048, 2048], w: [2048, 2048]

Sharding: data-parallel over tokens (1024 of 8192 per core), weight
replicated; each core computes the scale locally (no collectives —
the small-payload all-reduce floor is ~20us, worse than a local scan).

Weight traffic (the big change vs the two-stream predecessor): the f32
weight never ships. Quantization reads the fp16 copy of w^T directly
(fp16 rounding flips ~7e-5 of the ternary decisions near the
threshold; measured end-to-end rel err ~8.5e-3, well under the 2e-2
gate), and the scale is computed from a separate 4 MiB
stochastically-rounded fp8(e5m2) copy. SR makes the cast unbiased per
element, so mean(|w8|) matches mean(|w|) to ~1e-5 relative (plain RTN
fp8 has an ~8e-4 bias — too coarse); e5m2 keeps every |w| value in
the normal range, so even a subnormal-flushing engine path stays
exact. Per-core DMA drops from 36 MiB to 24 MiB:
fp8 4 + fp16 8 + x(bf16) 4 + out 8.

Timeline per core: the fp8 scan streams first on the sync DMA ring
(4 x 1 MiB transfers + a quarter-MiB head; big transfers because the
effective stream rate collapses below ~1 MiB) and is abs-reduced at
arrival by ACT (Abs + accum_out) and DVE (reduce, abs) on balanced
column spans; the scan ends ~28.5us and the scale chain lands ~30us.
Concurrent streams round-robin the SDMA engines at packet granularity
and would halve the scan's bandwidth, so the wh and late-x streams
are held back by GpSimd corner-write gates (tiny ops only — full-tile
GpSimd tensor ops are 5-30us, measured) and released into the scan's
tail. Quantization is the doubled ternary (exact in bf16, drain folds
scale/2): DVE path is 3 fused-op instructions (~2.45us/tile), and 7
of 16 tiles go through ACT Sign pairs (~3.9us/tile) whose combining
adds run on DVE just before the next DVE tile; the interleave was
chosen by simulating both pipelines against the PE's 1.7us/k-tile
phase-1 consumption (~2.4us total PE stall). The matmul runs m0/m1
k-outer across all 8 PSUM banks while quantization streams, then six
clean dense m-tiles. A train of bf16 dummy matmuls bridges the PE to
the scale so the HAM clock-gate never drops the PE to half clock and
the pbc broadcast fires immediately.

Matmul: bf16 x bf16 -> fp32 PSUM, K=2048 contracted in 16 accumulating
matmuls, N=512 per PSUM bank; psum drained ACT/DVE alternating with
scale/2 folded in.
"""

import sys

sys.path.insert(0, "/opt/trn_rl_repo")

import numpy as np

N_CORES = 8
B, S, D = 4, 2048, 2048        # x: [B, S, D]
OUT = 2048                     # out_features
TOK = B * S                    # 8192 tokens
TPC = TOK // N_CORES           # 1024 tokens per core
KT = D // 128                  # 16 K-tiles of 128
MT = TPC // 128                # 8 M-tiles per core
NT = OUT // 512                # 4 N-tiles of 512
N_ELEM = float(D * OUT)        # elements of w
EPS = 1e-5
M_P1 = 2                       # m-tiles in the k-outer first phase
SC = 4                         # fp8 scan chunks ([128, 8192] each)
N_WARM = 84                    # dummy matmuls bridging PE to the scale


def build_kernel():
    from concourse import bacc, tile, mybir

    f32 = mybir.dt.float32
    bf16 = mybir.dt.bfloat16
    fp16 = mybir.dt.float16
    fp8 = mybir.dt.float8e5
    Alu = mybir.AluOpType
    Act = mybir.ActivationFunctionType
    X = mybir.AxisListType.X

    nc = bacc.Bacc(None, target_bir_lowering=False)
    x_ext = nc.declare_dram_parameter("x", [TPC, D], bf16, isOutput=False)
    wh_ext = nc.declare_dram_parameter("wh", [D, OUT], fp16, isOutput=False)
    w8_ext = nc.declare_dram_parameter("w8", [SC * 128, 8192], fp8, isOutput=False)
    out_ext = nc.declare_dram_parameter("out", [TPC, OUT], f32, isOutput=True)

    with tile.TileContext(nc) as tc:
        with (
            tc.tile_pool(name="persist", bufs=1) as persist,
            tc.tile_pool(name="w8buf", bufs=4) as w8_pool,
            tc.tile_pool(name="whf", bufs=4) as whf_pool,
            tc.tile_pool(name="xbuf", bufs=8) as xbuf_pool,
            tc.tile_pool(name="sgn", bufs=7) as sgn_pool,
            tc.tile_pool(name="outp", bufs=2) as out_pool,
            tc.tile_pool(name="psum", bufs=8, space="PSUM") as psum_pool,
        ):
            wq = persist.tile([128, KT, OUT], bf16)      # quantized w^T (doubled)
            ones = persist.tile([128, 128], f32)
            ones_bf = persist.tile([128, 512], bf16)
            partials = persist.tile([128, 12], f32)      # ACT slots 0-5, DVE 6-11
            tot = persist.tile([128, 1], f32)
            scale_sb = persist.tile([128, 1], f32)
            t_pos = persist.tile([128, 1], f32)
            t_neg = persist.tile([128, 1], f32)
            s_half = persist.tile([128, 1], f32)

            nc.vector.memset(ones[:], 1.0)
            nc.vector.memset(ones_bf[:], 1.0)

            def x_dma(m, eng):
                xb = xbuf_pool.tile([128, KT, 128], bf16, tag="xbuf", name=f"xb{m}")
                eng.dma_start(
                    xb[:],
                    x_ext[m * 128 : (m + 1) * 128, :].rearrange(
                        "p (k c) -> p k c", k=KT
                    ),
                )
                return xb

            # ---- fp8 scan: 4 x 1 MiB transfers (bigger transfers run
            # ~340 GB/s vs ~210 at 0.5 MiB); chunk 0 additionally ships a
            # quarter-MiB head so the engines start ~2us earlier. Tile
            # dependencies are region-level, so ACT/DVE work on column
            # spans of each chunk as it lands; the split is ~54/46
            # ACT-heavy (Abs+accum 141 G elem/s vs DVE reduce 119).
            # Concurrent streams land on different logical queues which
            # the SDMA engines round-robin at packet granularity, so
            # everything else is queued or gated behind the scan. ----
            w8s = {}
            na = nd = 0
            for c in range(SC):
                w8 = w8_pool.tile([128, 8192], fp8, tag="w8", name=f"w8_{c}")
                if c == 0:
                    nc.sync.dma_start(w8[:, :2048], w8_ext[0:128, :2048])
                    nc.sync.dma_start(w8[:, 2048:], w8_ext[0:128, 2048:])
                    acts = [(0, 1024), (2048, 4096)]
                    dves = [(1024, 2048), (4096, 8192)]
                elif c == SC - 1:
                    nc.sync.dma_start(w8[:, :6144], w8_ext[c * 128 : (c + 1) * 128, :6144])
                    nc.sync.dma_start(w8[:, 6144:], w8_ext[c * 128 : (c + 1) * 128, 6144:])
                    acts = [(0, 4096), (6144, 7168)]
                    dves = [(4096, 6144), (7168, 8192)]
                else:
                    nc.sync.dma_start(w8[:], w8_ext[c * 128 : (c + 1) * 128, :])
                    acts, dves = [(0, 4096)], [(4096, 8192)]
                w8s[c] = w8
                for lo, hi in acts:
                    nc.scalar.activation(
                        w8[:, lo:hi], w8[:, lo:hi], Act.Abs,
                        accum_out=partials[:, na : na + 1],
                    )
                    na += 1
                for lo, hi in dves:
                    nc.vector.tensor_reduce(
                        partials[:, 6 + nd : 7 + nd], w8[:, lo:hi],
                        axis=X, op=Alu.add, apply_absolute_value=True,
                    )
                    nd += 1

            # ---- wh fp16 stream (quant source), 8 x 1 MiB pair transfers.
            # Only the FIRST transfer is gated (a GpSimd corner-write that
            # reads scan chunk 2 — idle engine, fires the moment that DMA
            # lands); the sync sequencer dispatches in program order, so
            # the rest queue behind it. The stream thus starts during the
            # scan's tail without competing with its body. x m0/m1 follow
            # the wh triggers on the same ring: their transfers round-robin
            # with the in-flight wh pairs and land ~26us, well before the
            # PE needs them at ~31us. ----
            whs = {}
            for j in range(KT // 2):
                wh = whf_pool.tile([128, 2, OUT], fp16, tag="whf", name=f"wh{j}")
                if j == 0:
                    nc.gpsimd.tensor_scalar(
                        wh[0:1, 0:1, 0:1], w8s[2][0:1, 0:1], 0.0, None, Alu.mult
                    )
                nc.sync.dma_start(
                    wh[:],
                    wh_ext[j * 256 : (j + 1) * 256, :].rearrange(
                        "(t p) o -> p t o", p=128
                    ),
                )
                whs[j] = wh

            xbufs = {}
            for m in range(M_P1):
                xbufs[m] = x_dma(m, nc.sync)

            # ---- PE warm train: keeps the HAM clock-gate at full rate and
            # the PE sequencer hot until the scale lands (bf16: a dummy is
            # 213 ns warm; fp32 would be 4x that and overshoot) ----
            warm = psum_pool.tile([128, 512], f32, tag="psum", name="warm")
            for i in range(N_WARM):
                nc.tensor.matmul(
                    warm[:], ones_bf[:, 0:128], ones_bf[:], start=True, stop=True
                )

            # ---- scale: sum partials, broadcast via ones-matmul; the
            # derived scalars go to three different engines so the chain
            # doesn't serialize on DVE ----
            nc.vector.tensor_reduce(tot[:], partials[:], axis=X, op=Alu.add)
            pbc = psum_pool.tile([128, 512], f32, tag="psum", name="pbc")
            nc.tensor.matmul(pbc[:, 0:1], ones[:], tot[:], start=True, stop=True)
            nc.vector.tensor_scalar(
                scale_sb[:], pbc[:, 0:1], 1.0 / N_ELEM, EPS, Alu.mult, Alu.max
            )
            nc.vector.tensor_scalar(t_pos[:], scale_sb[:], 1.0 / 3.0, None, Alu.mult)
            nc.scalar.activation(t_neg[:], scale_sb[:], Act.Copy, scale=-1.0 / 3.0)

            # remaining x m-tiles: sync ring behind the wh stream, released
            # by a corner-write that reads the last wh pair (they are first
            # needed at phase 2, ~10us after that pair lands, and must not
            # race the wh stream the PE is being fed from)
            for m in range(M_P1, MT):
                xb = xbuf_pool.tile([128, KT, 128], bf16, tag="xbuf", name=f"xb{m}")
                if m == M_P1:
                    nc.gpsimd.tensor_scalar(
                        xb[0:1, 0:1, 0:1], whs[5][0:1, 0:1, 0:1],
                        0.0, None, Alu.mult,
                    )
                nc.sync.dma_start(
                    xb[:],
                    x_ext[m * 128 : (m + 1) * 128, :].rearrange(
                        "p (k c) -> p k c", k=KT
                    ),
                )
                xbufs[m] = xb

            # ---- quantize one K-tile (doubled ternary {-2,0,2}, exact in
            # bf16; the drain folds in scale/2). DVE path is 3 fused-op
            # instructions (~2.45us/tile measured); five tiles go through
            # ACT Sign pairs (~3.9us/tile) whose combining adds run on DVE
            # one DVE-tile late, so neither engine ever stalls on the
            # other. Emission is interleaved k-order so tile-pool slots
            # never create cross-engine allocation cycles. ----
            ACT_TILES = (2, 4, 7, 9, 11, 13, 15)
            sgns = {}

            def quantize_act_signs(k):
                wt = whs[k // 2][:, k % 2, :]
                s1 = sgn_pool.tile([128, OUT], bf16, tag="sgn", name=f"s1_{k}")
                s2 = sgn_pool.tile([128, OUT], bf16, tag="sgn", name=f"s2_{k}")
                nc.scalar.activation(s1[:], wt[:], Act.Sign, bias=t_pos[:, 0:1])
                nc.scalar.activation(s2[:], wt[:], Act.Sign, bias=t_neg[:, 0:1])
                sgns[k] = (s1, s2)

            def combine_act(k):
                s1, s2 = sgns.pop(k)
                nc.vector.tensor_tensor(wq[:, k, :], s1[:], s2[:], Alu.add)

            def quantize_dve(k):
                wt = whs[k // 2][:, k % 2, :]
                neg = sgn_pool.tile([128, OUT], bf16, tag="sgn", name=f"n_{k}")
                nc.vector.tensor_scalar(
                    wq[:, k, :], wt[:], t_pos[:, 0:1], 2.0, Alu.is_gt, Alu.mult
                )
                nc.vector.tensor_scalar(
                    neg[:], wt[:], t_neg[:, 0:1], 2.0, Alu.is_lt, Alu.mult
                )
                nc.vector.tensor_tensor(
                    wq[:, k, :], wq[:, k, :], neg[:], Alu.subtract
                )

            pending = []
            for k in range(KT):
                if k in ACT_TILES:
                    quantize_act_signs(k)
                    pending.append(k)
                elif k == 0:
                    for lo, hi in ((0, OUT // 2), (OUT // 2, OUT)):
                        neg = sgn_pool.tile(
                            [128, hi - lo], bf16, tag="sgn", name=f"n0_{lo}"
                        )
                        wt = whs[0][:, 0, :]
                        nc.vector.tensor_scalar(
                            wq[:, 0, lo:hi], wt[:, lo:hi], t_pos[:, 0:1], 2.0,
                            Alu.is_gt, Alu.mult,
                        )
                        nc.vector.tensor_scalar(
                            neg[:], wt[:, lo:hi], t_neg[:, 0:1], 2.0,
                            Alu.is_lt, Alu.mult,
                        )
                        nc.vector.tensor_tensor(
                            wq[:, 0, lo:hi], wq[:, 0, lo:hi], neg[:], Alu.subtract
                        )
                    nc.vector.tensor_scalar(
                        s_half[:], scale_sb[:], 0.5, None, Alu.mult
                    )
                else:
                    while pending and pending[0] < k:
                        combine_act(pending.pop(0))
                    quantize_dve(k)
            for k in pending:
                combine_act(k)

            # ---- matmul: out[m,n] = sum_k x[k,m].T @ wq[k,n] ----
            def do_mtile(ms):
                psums = [
                    psum_pool.tile([128, 512], f32, tag="psum", name=f"ps{i}")
                    for i in range(NT * len(ms))
                ]
                for ki, k in enumerate(range(KT)):
                    for mi, m in enumerate(ms):
                        for n in range(NT):
                            nc.tensor.matmul(
                                psums[mi * NT + n][:],
                                xbufs[m][:, k, :],
                                wq[:, k, n * 512 : (n + 1) * 512],
                                start=(ki == 0),
                                stop=(ki == KT - 1),
                            )
                # psum drain alternates ACT/DVE so a tile's four copies
                # take ~2 serial slots instead of 4; the very last m-tile
                # drains in 256-col chunks so the final copy after the
                # final matmul is as short as possible
                for mi, m in enumerate(ms):
                    ot = out_pool.tile([128, OUT], f32, tag="outp", name=f"ot{m}")
                    step = 256 if m == MT - 1 else 512
                    for lo in range(0, OUT, step):
                        n = lo // 512
                        sl = ot[:, lo : lo + step]
                        ps = psums[mi * NT + n][:, lo - n * 512 : lo - n * 512 + step]
                        if (lo // step) % 2 == 0:
                            nc.scalar.activation(
                                sl, ps, Act.Copy, scale=s_half[:, 0:1]
                            )
                        else:
                            nc.vector.tensor_scalar(
                                sl, ps, s_half[:, 0:1], None, Alu.mult
                            )
                        nc.sync.dma_start(
                            out_ext[m * 128 : (m + 1) * 128, lo : lo + step],
                            sl,
                        )

            do_mtile(list(range(M_P1)))
            for m in range(M_P1, MT):
                do_mtile([m])

    nc.finalize()
    return nc


_NC_CACHE = None


def _sr_fp8_e5m2(w):
    """Stochastically-rounded cast to fp8 e5m2 (fixed seed, unbiased
    per element, so mean(|cast|) tracks mean(|w|) to ~1e-5 rel)."""
    import ml_dtypes

    rng = np.random.default_rng(0x5EED)
    xf = w.astype(np.float32)
    ax = np.abs(xf)
    e = np.floor(np.log2(np.maximum(ax, 1e-30)))
    min_norm = np.float32(2.0**-14)
    ulp = np.where(ax >= min_norm, 2.0 ** (e - 2), min_norm * 2.0**-2).astype(
        np.float32
    )
    lo = (np.floor(xf.astype(np.float64) / ulp) * ulp).astype(np.float32)
    p = ((xf - lo) / ulp).astype(np.float32)
    u = rng.random(xf.shape, dtype=np.float32)
    return (lo + ulp * (u < p).astype(np.float32)).astype(ml_dtypes.float8_e5m2)


def kernel(x, weight):
    global _NC_CACHE
    import ml_dtypes
    from concourse.bass_utils import run_bass_kernel_spmd

    x = np.asarray(x, dtype=np.float32).reshape(TOK, D)
    weight = np.asarray(weight, dtype=np.float32)
    wT = np.ascontiguousarray(weight.T)                      # [in, out] f32
    wh = wT.astype(np.float16)                               # quant source
    w8 = _sr_fp8_e5m2(wT).reshape(SC * 128, 8192)            # scale-only copy
    in_maps = []
    for i in range(N_CORES):
        shard_t = x[i * TPC : (i + 1) * TPC].T                      # [in, tok]
        tiled = (
            shard_t.reshape(KT, 128, MT, 128)
            .transpose(2, 1, 0, 3)
            .reshape(MT * 128, KT * 128)
        )
        in_maps.append(
            {"x": np.ascontiguousarray(tiled).astype(ml_dtypes.bfloat16),
             "wh": wh,
             "w8": w8}
        )

    if _NC_CACHE is None:
        _NC_CACHE = build_kernel()
    res = run_bass_kernel_spmd(_NC_CACHE, in_maps, core_ids=list(range(N_CORES)))
    outs = [res.results[i]["out"] for i in range(N_CORES)]
    return np.concatenate(outs, axis=0).reshape(B, S, OUT).astype(np.float32)


# revision 35
# speedup vs baseline: 1.0087x; 1.0087x over previous
"""BitLinear (BitNet 1.58-bit ternary) distributed Trainium2 kernel.

Reference semantics:
    scale = max(mean(|w|), 1e-5)
    w_q   = sign(w) * (|w| > scale/3)          # ternary {-1, 0, 1}
    out   = (x @ w_q.T) * scale                # x: [4, 2048, 2048], w: [2048, 2048]

Sharding: data-parallel over tokens (1024 of 8192 per core), weight
replicated; each core computes the scale locally (no collectives —
the small-payload all-reduce floor is ~20us, worse than a local scan).

Weight traffic (the big change vs the two-stream predecessor): the f32
weight never ships. Quantization reads the fp16 copy of w^T directly
(fp16 rounding flips ~7e-5 of the ternary decisions near the
threshold; measured end-to-end rel err ~8.5e-3, well under the 2e-2
gate), and the scale is computed from a separate 4 MiB
stochastically-rounded fp8(e5m2) copy. SR makes the cast unbiased per
element, so mean(|w8|) matches mean(|w|) to ~1e-5 relative (plain RTN
fp8 has an ~8e-4 bias — too coarse); e5m2 keeps every |w| value in
the normal range, so even a subnormal-flushing engine path stays
exact. Per-core DMA drops from 36 MiB to 24 MiB:
fp8 4 + fp16 8 + x(bf16) 4 + out 8.

Timeline per core: the fp8 scan streams first on the sync DMA ring
(4 x 1 MiB transfers + a quarter-MiB head; big transfers because the
effective stream rate collapses below ~1 MiB) and is abs-reduced at
arrival by ACT (Abs + accum_out) and DVE (reduce, abs) on balanced
column spans; the scan ends ~28.5us and the scale chain lands ~30us.
Concurrent streams round-robin the SDMA engines at packet granularity
and would halve the scan's bandwidth, so the wh and late-x streams
are held back by GpSimd corner-write gates (tiny ops only — full-tile
GpSimd tensor ops are 5-30us, measured) and released into the scan's
tail. Quantization is the doubled ternary (exact in bf16, drain folds
scale/2): DVE path is 3 fused-op instructions (~2.45us/tile), and 7
of 16 tiles go through ACT Sign pairs (~3.9us/tile) whose combining
adds run on DVE just before the next DVE tile; the interleave was
chosen by simulating both pipelines against the PE's 1.7us/k-tile
phase-1 consumption (~2.4us total PE stall). The matmul runs m0/m1
k-outer across all 8 PSUM banks while quantization streams, then six
clean dense m-tiles. A train of bf16 dummy matmuls bridges the PE to
the scale so the HAM clock-gate never drops the PE to half clock and
the pbc broadcast fires immediately.

Matmul: bf16 x bf16 -> fp32 PSUM, K=2048 contracted in 16 accumulating
matmuls, N=512 per PSUM bank; psum drained ACT/DVE alternating with
scale/2 folded in.
"""

import sys

sys.path.insert(0, "/opt/trn_rl_repo")

import numpy as np

N_CORES = 8
B, S, D = 4, 2048, 2048        # x: [B, S, D]
OUT = 2048                     # out_features
TOK = B * S                    # 8192 tokens
TPC = TOK // N_CORES           # 1024 tokens per core
KT = D // 128                  # 16 K-tiles of 128
MT = TPC // 128                # 8 M-tiles per core
NT = OUT // 512                # 4 N-tiles of 512
N_ELEM = float(D * OUT)        # elements of w
EPS = 1e-5
M_P1 = 2                       # m-tiles in the k-outer first phase
SC = 4                         # fp8 scan chunks ([128, 8192] each)
N_WARM = 84                    # dummy matmuls bridging PE to the scale


def build_kernel():
    from concourse import bacc, tile, mybir

    f32 = mybir.dt.float32
    bf16 = mybir.dt.bfloat16
    fp16 = mybir.dt.float16
    fp8 = mybir.dt.float8e5
    Alu = mybir.AluOpType
    Act = mybir.ActivationFunctionType
    X = mybir.AxisListType.X

    nc = bacc.Bacc(None, target_bir_lowering=False)
    x_ext = nc.declare_dram_parameter("x", [TPC, D], bf16, isOutput=False)
    wh_ext = nc.declare_dram_parameter("wh", [D, OUT], fp16, isOutput=False)
    w8_ext = nc.declare_dram_parameter("w8", [SC * 128, 8192], fp8, isOutput=False)
    out_ext = nc.declare_dram_parameter("out", [TPC, OUT], f32, isOutput=True)

    with tile.TileContext(nc) as tc:
        with (
            tc.tile_pool(name="persist", bufs=1) as persist,
            tc.tile_pool(name="w8buf", bufs=4) as w8_pool,
            tc.tile_pool(name="whf", bufs=4) as whf_pool,
            tc.tile_pool(name="xbuf", bufs=8) as xbuf_pool,
            tc.tile_pool(name="sgn", bufs=7) as sgn_pool,
            tc.tile_pool(name="outp", bufs=2) as out_pool,
            tc.tile_pool(name="psum", bufs=8, space="PSUM") as psum_pool,
        ):
            wq = persist.tile([128, KT, OUT], bf16)      # quantized w^T (doubled)
            ones = persist.tile([128, 128], f32)
            ones_bf = persist.tile([128, 512], bf16)
            partials = persist.tile([128, 12], f32)      # ACT slots 0-5, DVE 6-11
            tot = persist.tile([128, 1], f32)
            scale_sb = persist.tile([128, 1], f32)
            t_pos = persist.tile([128, 1], f32)
            t_neg = persist.tile([128, 1], f32)
            s_half = persist.tile([128, 1], f32)

            nc.vector.memset(ones[:], 1.0)
            nc.vector.memset(ones_bf[:], 1.0)

            def x_dma(m, eng):
                xb = xbuf_pool.tile([128, KT, 128], bf16, tag="xbuf", name=f"xb{m}")
                eng.dma_start(
                    xb[:],
                    x_ext[m * 128 : (m + 1) * 128, :].rearrange(
                        "p (k c) -> p k c", k=KT
                    ),
                )
                return xb

            # ---- fp8 scan: 4 x 1 MiB transfers (bigger transfers run
            # ~340 GB/s vs ~210 at 0.5 MiB); chunk 0 additionally ships a
            # quarter-MiB head so the engines start ~2us earlier. Tile
            # dependencies are region-level, so ACT/DVE work on column
            # spans of each chunk as it lands; the split is ~54/46
            # ACT-heavy (Abs+accum 141 G elem/s vs DVE reduce 119).
            # Concurrent streams land on different logical queues which
            # the SDMA engines round-robin at packet granularity, so
            # everything else is queued or gated behind the scan. ----
            w8s = {}
            na = nd = 0
            for c in range(SC):
                w8 = w8_pool.tile([128, 8192], fp8, tag="w8", name=f"w8_{c}")
                if c == 0:
                    nc.sync.dma_start(w8[:, :2048], w8_ext[0:128, :2048])
                    nc.sync.dma_start(w8[:, 2048:], w8_ext[0:128, 2048:])
                    acts = [(0, 1024), (2048, 5120)]
                    dves = [(1024, 2048), (5120, 8192)]
                elif c == SC - 1:
                    nc.sync.dma_start(w8[:, :6144], w8_ext[c * 128 : (c + 1) * 128, :6144])
                    nc.sync.dma_start(w8[:, 6144:], w8_ext[c * 128 : (c + 1) * 128, 6144:])
                    acts = [(0, 4096), (6144, 7168)]
                    dves = [(4096, 6144), (7168, 8192)]
                else:
                    nc.sync.dma_start(w8[:], w8_ext[c * 128 : (c + 1) * 128, :])
                    acts, dves = [(0, 4096)], [(4096, 8192)]
                w8s[c] = w8
                for lo, hi in acts:
                    nc.scalar.activation(
                        w8[:, lo:hi], w8[:, lo:hi], Act.Abs,
                        accum_out=partials[:, na : na + 1],
                    )
                    na += 1
                for lo, hi in dves:
                    nc.vector.tensor_reduce(
                        partials[:, 6 + nd : 7 + nd], w8[:, lo:hi],
                        axis=X, op=Alu.add, apply_absolute_value=True,
                    )
                    nd += 1

            # ---- wh fp16 stream (quant source), 8 x 1 MiB pair transfers.
            # Only the FIRST transfer is gated (a GpSimd corner-write that
            # reads scan chunk 2 — idle engine, fires the moment that DMA
            # lands); the sync sequencer dispatches in program order, so
            # the rest queue behind it. The stream thus starts during the
            # scan's tail without competing with its body. x m0/m1 follow
            # the wh triggers on the same ring: their transfers round-robin
            # with the in-flight wh pairs and land ~26us, well before the
            # PE needs them at ~31us. ----
            whs = {}
            for j in range(KT // 2):
                wh = whf_pool.tile([128, 2, OUT], fp16, tag="whf", name=f"wh{j}")
                if j == 0:
                    nc.gpsimd.tensor_scalar(
                        wh[0:1, 0:1, 0:1], w8s[2][0:1, 0:1], 0.0, None, Alu.mult
                    )
                nc.sync.dma_start(
                    wh[:],
                    wh_ext[j * 256 : (j + 1) * 256, :].rearrange(
                        "(t p) o -> p t o", p=128
                    ),
                )
                whs[j] = wh

            xbufs = {}
            for m in range(M_P1):
                xbufs[m] = x_dma(m, nc.sync)

            # ---- PE warm train: keeps the HAM clock-gate at full rate and
            # the PE sequencer hot until the scale lands (bf16: a dummy is
            # 213 ns warm; fp32 would be 4x that and overshoot) ----
            warm = psum_pool.tile([128, 512], f32, tag="psum", name="warm")
            for i in range(N_WARM):
                nc.tensor.matmul(
                    warm[:], ones_bf[:, 0:128], ones_bf[:], start=True, stop=True
                )

            # ---- scale: sum partials, broadcast via ones-matmul; the
            # derived scalars go to three different engines so the chain
            # doesn't serialize on DVE ----
            nc.vector.tensor_reduce(tot[:], partials[:], axis=X, op=Alu.add)
            pbc = psum_pool.tile([128, 512], f32, tag="psum", name="pbc")
            nc.tensor.matmul(pbc[:, 0:1], ones[:], tot[:], start=True, stop=True)
            nc.vector.tensor_scalar(
                scale_sb[:], pbc[:, 0:1], 1.0 / N_ELEM, EPS, Alu.mult, Alu.max
            )
            nc.vector.tensor_scalar(t_pos[:], scale_sb[:], 1.0 / 3.0, None, Alu.mult)
            nc.scalar.activation(t_neg[:], scale_sb[:], Act.Copy, scale=-1.0 / 3.0)

            # remaining x m-tiles: sync ring behind the wh stream, released
            # by a corner-write that reads the last wh pair (they are first
            # needed at phase 2, ~10us after that pair lands, and must not
            # race the wh stream the PE is being fed from)
            for m in range(M_P1, MT):
                xb = xbuf_pool.tile([128, KT, 128], bf16, tag="xbuf", name=f"xb{m}")
                if m == M_P1:
                    nc.gpsimd.tensor_scalar(
                        xb[0:1, 0:1, 0:1], whs[5][0:1, 0:1, 0:1],
                        0.0, None, Alu.mult,
                    )
                nc.sync.dma_start(
                    xb[:],
                    x_ext[m * 128 : (m + 1) * 128, :].rearrange(
                        "p (k c) -> p k c", k=KT
                    ),
                )
                xbufs[m] = xb

            # ---- quantize one K-tile (doubled ternary {-2,0,2}, exact in
            # bf16; the drain folds in scale/2). DVE path is 3 fused-op
            # instructions (~2.45us/tile measured); five tiles go through
            # ACT Sign pairs (~3.9us/tile) whose combining adds run on DVE
            # one DVE-tile late, so neither engine ever stalls on the
            # other. Emission is interleaved k-order so tile-pool slots
            # never create cross-engine allocation cycles. ----
            ACT_TILES = (2, 4, 7, 9, 11, 13, 15)
            sgns = {}

            def quantize_act_signs(k):
                wt = whs[k // 2][:, k % 2, :]
                s1 = sgn_pool.tile([128, OUT], bf16, tag="sgn", name=f"s1_{k}")
                s2 = sgn_pool.tile([128, OUT], bf16, tag="sgn", name=f"s2_{k}")
                nc.scalar.activation(s1[:], wt[:], Act.Sign, bias=t_pos[:, 0:1])
                nc.scalar.activation(s2[:], wt[:], Act.Sign, bias=t_neg[:, 0:1])
                sgns[k] = (s1, s2)

            def combine_act(k):
                s1, s2 = sgns.pop(k)
                nc.vector.tensor_tensor(wq[:, k, :], s1[:], s2[:], Alu.add)

            def quantize_dve(k):
                wt = whs[k // 2][:, k % 2, :]
                neg = sgn_pool.tile([128, OUT], bf16, tag="sgn", name=f"n_{k}")
                nc.vector.tensor_scalar(
                    wq[:, k, :], wt[:], t_pos[:, 0:1], 2.0, Alu.is_gt, Alu.mult
                )
                nc.vector.tensor_scalar(
                    neg[:], wt[:], t_neg[:, 0:1], 2.0, Alu.is_lt, Alu.mult
                )
                nc.vector.tensor_tensor(
                    wq[:, k, :], wq[:, k, :], neg[:], Alu.subtract
                )

            pending = []
            for k in range(KT):
                if k in ACT_TILES:
                    quantize_act_signs(k)
                    pending.append(k)
                elif k == 0:
                    for lo, hi in ((0, OUT // 2), (OUT // 2, OUT)):
                        neg = sgn_pool.tile(
                            [128, hi - lo], bf16, tag="sgn", name=f"n0_{lo}"
                        )
                        wt = whs[0][:, 0, :]
                        nc.vector.tensor_scalar(
                            wq[:, 0, lo:hi], wt[:, lo:hi], t_pos[:, 0:1], 2.0,
                            Alu.is_gt, Alu.mult,
                        )
                        nc.vector.tensor_scalar(
                            neg[:], wt[:, lo:hi], t_neg[:, 0:1], 2.0,
                            Alu.is_lt, Alu.mult,
                        )
                        nc.vector.tensor_tensor(
                            wq[:, 0, lo:hi], wq[:, 0, lo:hi], neg[:], Alu.subtract
                        )
                    nc.vector.tensor_scalar(
                        s_half[:], scale_sb[:], 0.5, None, Alu.mult
                    )
                else:
                    while pending and pending[0] < k:
                        combine_act(pending.pop(0))
                    quantize_dve(k)
            for k in pending:
                combine_act(k)

            # ---- matmul: out[m,n] = sum_k x[k,m].T @ wq[k,n] ----
            def do_mtile(ms):
                psums = [
                    psum_pool.tile([128, 512], f32, tag="psum", name=f"ps{i}")
                    for i in range(NT * len(ms))
                ]
                for ki, k in enumerate(range(KT)):
                    for mi, m in enumerate(ms):
                        for n in range(NT):
                            nc.tensor.matmul(
                                psums[mi * NT + n][:],
                                xbufs[m][:, k, :],
                                wq[:, k, n * 512 : (n + 1) * 512],
                                start=(ki == 0),
                                stop=(ki == KT - 1),
                            )
                # psum drain alternates ACT/DVE so a tile's four copies
                # take ~2 serial slots instead of 4 (finer chunks measured
                # slower: each extra out-DMA costs ~0.6us of dispatch)
                for mi, m in enumerate(ms):
                    ot = out_pool.tile([128, OUT], f32, tag="outp", name=f"ot{m}")
                    for n in range(NT):
                        sl = ot[:, n * 512 : (n + 1) * 512]
                        if n % 2 == 0:
                            nc.scalar.activation(
                                sl, psums[mi * NT + n][:], Act.Copy,
                                scale=s_half[:, 0:1],
                            )
                        else:
                            nc.vector.tensor_scalar(
                                sl, psums[mi * NT + n][:], s_half[:, 0:1],
                                None, Alu.mult,
                            )
                        nc.sync.dma_start(
                            out_ext[m * 128 : (m + 1) * 128, n * 512 : (n + 1) * 512],
                            sl,
                        )

            do_mtile(list(range(M_P1)))
            for m in range(M_P1, MT):
                do_mtile([m])

    nc.finalize()
    return nc


_NC_CACHE = None


def _sr_fp8_e5m2(w):
    """Stochastically-rounded cast to fp8 e5m2 (fixed seed, unbiased
    per element, so mean(|cast|) tracks mean(|w|) to ~1e-5 rel)."""
    import ml_dtypes

    rng = np.random.default_rng(0x5EED)
    xf = w.astype(np.float32)
    ax = np.abs(xf)
    e = np.floor(np.log2(np.maximum(ax, 1e-30)))
    min_norm = np.float32(2.0**-14)
    ulp = np.where(ax >= min_norm, 2.0 ** (e - 2), min_norm * 2.0**-2).astype(
        np.float32
    )
    lo = (np.floor(xf.astype(np.float64) / ulp) * ulp).astype(np.float32)
    p = ((xf - lo) / ulp).astype(np.float32)
    u = rng.random(xf.shape, dtype=np.float32)
    return (lo + ulp * (u < p).astype(np.float32)).astype(ml_dtypes.float8_e5m2)


def kernel(x, weight):
    global _NC_CACHE
    import ml_dtypes
    from concourse.bass_utils import run_bass_kernel_spmd

    x = np.asarray(x, dtype=np.float32).reshape(TOK, D)
    weight = np.asarray(weight, dtype=np.float32)
    wT = np.ascontiguousarray(weight.T)                      # [in, out] f32
    wh = wT.astype(np.float16)                               # quant source
    w8 = _sr_fp8_e5m2(wT).reshape(SC * 128, 8192)            # scale-only copy
    in_maps = []
    for i in range(N_CORES):
        shard_t = x[i * TPC : (i + 1) * TPC].T                      # [in, tok]
        tiled = (
            shard_t.reshape(KT, 128, MT, 128)
            .transpose(2, 1, 0, 3)
            .reshape(MT * 128, KT * 128)
        )
        in_maps.append(
            {"x": np.ascontiguousarray(tiled).astype(ml_dtypes.bfloat16),
             "wh": wh,
             "w8": w8}
        )

    if _NC_CACHE is None:
        _NC_CACHE = build_kernel()
    res = run_bass_kernel_spmd(_NC_CACHE, in_maps, core_ids=list(range(N_CORES)))
    outs = [res.results[i]["out"] for i in range(N_CORES)]
    return np.concatenate(outs, axis=0).reshape(B, S, OUT).astype(np.float32)
